# revision 54
# baseline (speedup 1.0000x reference)
"""BBox window attention kernel for 8 TRN2 NeuronCores — streaming schedule.

Sharding: data-parallel over batch B=8 -> one batch element per core.
Each core computes the full attention for its batch element; no collectives.

v2: single streaming pipeline. x is loaded per 512-token block; each block's
cast/transpose/qkv-projection/s0 work is emitted as small "filler quanta"
interleaved between window-attention front/back steps, so the per-iteration
softmax chain (exp -> reduce -> recip -> gpsimd normalize) is hidden behind
projection matmuls and the PE never starves. Output projection tiles of
supergroup g ride as filler inside supergroup g+1.

Per-core math (all matmuls bf16 with f32 PSUM accumulation) is identical to
v1: feature-major q/k, token-major v (shifted by 1), global token via exp
without max-subtraction, windows in 16-window supergroups with PSUM
tile_position row discipline (row-0 pools vs row-64 pool).
"""

import sys

for _p in ("/opt/trn_rl_repo",):
    if _p not in sys.path:
        sys.path.insert(0, _p)

import numpy as np

import concourse.bass as bass
import concourse.tile as tile
from concourse import bacc, mybir
from concourse.bass_utils import run_bass_kernel_spmd
from concourse.masks import make_identity

F32 = mybir.dt.float32
BF16 = mybir.dt.bfloat16

B, T_FULL, D = 8, 4097, 512
H, WIN, d_head = 8, 64, 64
N_CORES = 8
CH = 4          # head-pair chunks (128 features each)
KC = 4          # contraction chunks of 128 over D
BLK = 512       # token block size (one PSUM bank at f32)
SCALE = float(d_head) ** -0.5


def _emit(nc, tc, x_d, wqkv_d, wout_d, out_d, T):
    TW = T - 1                 # window tokens
    NW = TW // WIN             # number of windows (64)
    assert NW % 16 == 0
    WG2 = NW // 16             # supergroups of 16 windows (4)
    NBLK = TW // BLK           # 8 full blocks; block NBLK is the 1-token tail
    VT = TW // 128             # v tiles (tokens 1..TW)
    TQ = (T + 127) // 128      # output tiles of 128 tokens

    def pool(name, **kw):
        return tc.tile_pool(name=name, **kw)

    with pool("persist", bufs=1) as persist, \
         pool("xstage", bufs=2) as xstage, \
         pool("stats", bufs=4) as stats, \
         pool("pp", bufs=4) as pp, \
         pool("osb", bufs=5) as posb, \
         pool("psum_w0", bufs=3, space="PSUM") as pwin, \
         pool("psum_pr", bufs=3, space="PSUM") as pproj, \
         pool("psum_r64", bufs=2, space="PSUM") as pr64:

        # PSUM discipline (hardware-validated): all matmul groups landing in
        # one physical bank must share the same tile_position ROW (= lhsT/rhs
        # partition base).  pwin/pproj host row-0 groups only; pr64 hosts
        # row-64 groups (odd head-half S tiles / odd window-parity O tiles).

        ident = persist.tile([128, 128], BF16)

        wqkv_sb = persist.tile([128, KC, 3 * D], BF16)
        wout_sb = persist.tile([128, KC, D], BF16)
        qT = persist.tile([128, CH, T], BF16)
        kT = persist.tile([128, CH, T], BF16)
        v_sb = persist.tile([128, VT, D], BF16)
        v0_sb = persist.tile([1, D], BF16)
        q0all = persist.tile([128, CH, 8], BF16)
        P0T_sb = persist.tile([128, VT, 8], BF16)
        p00_sb = persist.tile([1, 8], BF16)
        ones_col = persist.tile([128, 1], BF16)
        ones_row = persist.tile([1, 128], BF16)
        zinv_sb = persist.tile([1, 8], F32)
        zinv_bf = persist.tile([1, 8], BF16)
        zb_sb = persist.tile([128, 8], F32)
        attnT = persist.tile([128, CH, T], BF16)

        st = {}  # per-block tile handles

        # ---------------- projection quanta ----------------

        def dma_x(j):
            if j < NBLK:
                xs = xstage.tile([128, 4, BLK], F32, tag="xs", name=f"xs{j}")
                if j == 0:
                    # quarters so block 0's cast/transposes start ~2.2us earlier
                    for qq in range(4):
                        nc.sync.dma_start(
                            out=xs[:, qq:qq + 1, :],
                            in_=x_d[128 * qq:128 * (qq + 1), :].rearrange(
                                "(a p) e -> p a e", p=128))
                elif j == 1:
                    # halves, interleaved into the w_qkv DMA stream
                    st[("xsh", 0)] = lambda xs=xs: nc.sync.dma_start(
                        out=xs[:, 0:2, :],
                        in_=x_d[BLK:BLK + 256, :].rearrange(
                            "(a p) e -> p a e", p=128))
                    st[("xsh", 1)] = lambda xs=xs: nc.sync.dma_start(
                        out=xs[:, 2:4, :],
                        in_=x_d[BLK + 256:2 * BLK, :].rearrange(
                            "(a p) e -> p a e", p=128))
                else:
                    nc.sync.dma_start(
                        out=xs[:, :, :],
                        in_=x_d[BLK * j:BLK * (j + 1), :].rearrange(
                            "(a p) e -> p a e", p=128),
                    )
            else:  # tail: token T-1 (shares the weight-staging slots)
                xs = xstage.tile([1, D], F32, tag="wst", bufs=3, name="xs_t")
                nc.sync.dma_start(out=xs[:, :], in_=x_d[T - 1:T, :])
            st[("xs", j)] = xs

        def cast_x(j):
            # f32 -> bf16 cast.  Prelude blocks (0-2) go on DVE/ACT (idle
            # there); later blocks go on GpSimd in two halves, keeping
            # DVE/ACT free for PSUM drains while Pool normalizes slot in
            # between the halves.
            xs = st.pop(("xs", j))
            if j < NBLK:
                xc = xstage.tile([128, 4, BLK], BF16, tag="xc", name=f"xc{j}")
                if j == 0:
                    for qq in range(4):
                        nc.vector.tensor_copy(xc[:, qq, :], xs[:, qq, :])
                elif j == 1:
                    nc.vector.tensor_copy(xc[:, 0:2, :], xs[:, 0:2, :])
                    nc.vector.tensor_copy(xc[:, 2:4, :], xs[:, 2:4, :])
                elif j == 2:
                    # prelude gap: DVE+ACT halves run in parallel while the
                    # window pipeline hasn't started yet
                    nc.vector.tensor_copy(xc[:, 0:2, :], xs[:, 0:2, :])
                    nc.scalar.copy(xc[:, 2:4, :], xs[:, 2:4, :])
                else:
                    nc.gpsimd.tensor_copy(xc[:, 0:2, :], xs[:, 0:2, :])
                    nc.gpsimd.tensor_copy(xc[:, 2:4, :], xs[:, 2:4, :])
            else:
                xc = xstage.tile([1, D], BF16, tag="xc", name="xc_t")
                nc.vector.tensor_copy(xc[:, :], xs[:, :])
            st[("xc", j)] = xc

        def transp(j, jj):
            """Transpose token tile jj (128 tokens) of block j into xT(j).

            xT blocks have 513 columns: col 512 (= next block's first token)
            is written by the next block's jj=0 call, so v tiles never span
            two xT tiles.
            """
            if j == NBLK:  # tail token: fills col 512 of block NBLK-1 only
                xc = st[("xc", j)]
                # inner dim 2 keeps each kc-slice 4-byte aligned in PSUM
                # (walrus requires 4B-aligned matmul outputs)
                tp = pproj.tile([128, KC, 2], BF16, tag="proj", name="tp_t")
                for kc in range(KC):
                    nc.tensor.transpose(
                        tp[:, kc, 0:1], xc[:, 128 * kc:128 * (kc + 1)],
                        ident[0:1, 0:1])
                nc.vector.tensor_copy(st[("xT", NBLK - 1)][:, :, BLK:BLK + 1],
                                      tp[:, :, 0:1])
                return
            xc = st[("xc", j)]
            if jj == 0:
                xT = xstage.tile([128, KC, BLK + 1], BF16, tag="xT",
                                 bufs=2, name=f"xT{j}")
                st[("xT", j)] = xT
            xT = st[("xT", j)]
            tp = pproj.tile([128, KC, 128], BF16, tag="proj", name="tp")
            for kc in range(KC):
                nc.tensor.transpose(
                    tp[:, kc, :], xc[:, jj, 128 * kc:128 * (kc + 1)],
                    ident[:, :])
            dst = xT[:, :, 128 * jj:128 * (jj + 1)]
            if jj % 2 == 0:
                nc.vector.tensor_copy(dst, tp[:, :, :])
            else:
                nc.scalar.copy(dst, tp[:, :, :])
            if jj == 0 and j > 0:
                # previous block's overlap column (token BLK*j)
                nc.vector.tensor_copy(st[("xT", j - 1)][:, :, BLK:BLK + 1],
                                      tp[:, :, 0:1])

        def jb_proj(j, jjb):
            """q/k feature block jjb (0..3 -> qT chunk, 4..7 -> kT chunk)."""
            c0 = BLK * j
            w = min(BLK, T - c0)
            ps = pproj.tile([128, BLK], F32, tag="proj", name="psjb")
            for kc in range(KC):
                if j < NBLK:
                    rhs = st[("xT", j)][:, kc, 0:w]
                else:  # tail token lives in block NBLK-1's overlap column
                    rhs = st[("xT", NBLK - 1)][:, kc, BLK:BLK + w]
                nc.tensor.matmul(
                    ps[:, :w],
                    wqkv_sb[:, kc, 128 * jjb:128 * (jjb + 1)],
                    rhs,
                    start=(kc == 0),
                    stop=(kc == KC - 1),
                )
            if jjb < 4:
                dst = qT[:, jjb, c0:c0 + w]
            else:
                dst = kT[:, jjb - 4, c0:c0 + w]
            if jjb % 2 == 0:
                nc.vector.tensor_copy(dst, ps[:, :w])
            else:
                nc.scalar.copy(dst, ps[:, :w])

        def v_proj(vt):
            """v tile vt: tokens 1+128vt .. 129+128vt (within xT block a)."""
            a = (128 * vt) // BLK
            off = 1 + 128 * vt - BLK * a
            xT = st[("xT", a)]
            ps = pproj.tile([128, D], F32, tag="proj", name="psv")
            for kc in range(KC):
                nc.tensor.matmul(
                    ps[:, :],
                    xT[:, kc, off:off + 128],
                    wqkv_sb[:, kc, 2 * D:3 * D],
                    start=(kc == 0),
                    stop=(kc == KC - 1),
                )
            if vt % 2 == 0:
                nc.vector.tensor_copy(v_sb[:, vt, :], ps[:, :])
            else:
                nc.scalar.copy(v_sb[:, vt, :], ps[:, :])

        def v0_proj():
            xT = st[("xT", 0)]
            ps = pproj.tile([1, D], F32, tag="proj", name="psv0")
            for kc in range(KC):
                nc.tensor.matmul(
                    ps[:, :], xT[:, kc, 0:1], wqkv_sb[:, kc, 2 * D:3 * D],
                    start=(kc == 0), stop=(kc == KC - 1))
            nc.vector.tensor_copy(v0_sb[:, :], ps[:, :])

        def build_q0all():
            # q0all column h holds q0 of head h only in head h's partition
            # range of its chunk and zeros elsewhere, so the four chunk
            # matmuls of s0 accumulate cleanly.
            nc.vector.memset(q0all[:, :, :], 0.0)
            for h in range(H):
                r0 = 64 * (h % 2)
                nc.vector.tensor_copy(
                    q0all[r0:r0 + 64, h // 2, h:h + 1],
                    qT[r0:r0 + 64, h // 2, 0:1])

        def s00_tok0():
            """Global-token score vs token 0, token-major: out [1, 8]."""
            ps = pproj.tile([1, 8], F32, tag="proj", name="ps00")
            for c in range(CH):
                nc.tensor.matmul(ps[:, :], kT[:, c, 0:1], q0all[:, c, :],
                                 start=(c == 0), stop=(c == CH - 1))
            nc.scalar.activation(p00_sb[:, :], ps[:, :],
                                 mybir.ActivationFunctionType.Exp,
                                 bias=0.0, scale=SCALE)

        def s0_quad(qd):
            """Global-token probs for 4 token tiles (128 tokens each),
            token-major: matmul N=8 per (tile, c) instead of N=512, and the
            exp'd result IS P0T (no transposes, no overlap columns)."""
            ps = pproj.tile([128, 4, 8], F32, tag="proj", name=f"ps0q{qd}")
            for i in range(4):
                t0 = 1 + 128 * (4 * qd + i)
                for c in range(CH):
                    nc.tensor.matmul(ps[:, i, :], kT[:, c, t0:t0 + 128],
                                     q0all[:, c, :],
                                     start=(c == 0), stop=(c == CH - 1))
            nc.scalar.activation(
                P0T_sb[:, 4 * qd:4 * qd + 4, :].rearrange("p a b -> p (a b)"),
                ps[:, :, :].rearrange("p a b -> p (a b)"),
                mybir.ActivationFunctionType.Exp, bias=0.0, scale=SCALE)

        def z_sum():
            """Softmax denominator part 1: ones-matmul over P0T tiles, then
            recip+cast on DVE (give PE filler before calling z_fin)."""
            zps = pproj.tile([1, 8], F32, tag="proj", name="zps")
            nc.tensor.matmul(zps[:, :], ones_col[0:1, 0:1], p00_sb[:, :],
                             start=True, stop=False)
            for vt in range(VT):
                nc.tensor.matmul(zps[:, :], ones_col[:, :], P0T_sb[:, vt, :],
                                 start=False, stop=(vt == VT - 1))
            nc.vector.reciprocal(zinv_sb[:, :], zps[:, :])
            nc.vector.tensor_copy(zinv_bf[:, :], zinv_sb[:, :])

        def z_fin():
            """Part 2: partition-broadcast 1/Z (outer product with ones)."""
            zbps = pproj.tile([128, 8], F32, tag="proj", name="zbps")
            nc.tensor.matmul(zbps[:, :], ones_row[:, :], zinv_bf[:, :],
                             start=True, stop=True)
            nc.vector.tensor_copy(zb_sb[:, :], zbps[:, :])

        def o0_accum():
            """out0 via feature-major accumulation: lhsT = v tile (tokens on
            partitions), rhs = P0T column pair -> [128 feats, 2] per chunk;
            N=2 matmuls make this ~free on the PE."""
            ops = pproj.tile([128, 4, 2], F32, tag="proj", name="o0ps")
            for c in range(CH):
                nc.tensor.matmul(ops[:, c, :],
                                 v0_sb[0:1, 128 * c:128 * (c + 1)],
                                 p00_sb[0:1, 2 * c:2 * c + 2],
                                 start=True, stop=False)
                for vt in range(VT):
                    nc.tensor.matmul(ops[:, c, :],
                                     v_sb[:, vt, 128 * c:128 * (c + 1)],
                                     P0T_sb[:, vt, 2 * c:2 * c + 2],
                                     start=False, stop=(vt == VT - 1))
            # normalize + scatter into attnT column 0 (diagonal strips)
            for c in range(CH):
                for r in range(2):
                    h = 2 * c + r
                    nc.scalar.activation(
                        attnT[64 * r:64 * r + 64, c, 0:1],
                        ops[64 * r:64 * r + 64, c, r:r + 1],
                        mybir.ActivationFunctionType.Identity,
                        bias=0.0, scale=zb_sb[64 * r:64 * r + 64, h:h + 1])

        # ---------------- window attention ----------------
        # Window wj (0..15 within a 16-window supergroup) maps to bits
        # (u, b1, s2) = (wj&1, (wj>>1)&1, wj>>2 in 0..3).  Layouts keep
        # every matmul's lhsT/rhs partition base equal and the
        # tile_position row fixed per PSUM tile (hardware requirement):
        #   S tile (per head-half r):  [64*b1 + q, slot=2*s2+u, k]
        #   PT (transposed P):         [64*u + k, slab=4*r+s2, 64*b1 + q]
        #   O tile (per parity u):     [64*r + e, slot=2*s2+b1, q]

        def win_front(wg2, c):
            """S matmuls + softmax for one iteration; returns P tiles."""
            P_sb = [None, None]
            for r in range(2):
                sp = (pwin if r == 0 else pr64).tile(
                    [128, 8, WIN], F32, tag=("big" if r == 0 else "r64"),
                    name="sp")
                for wj in range(16):
                    u, b1, s2 = wj & 1, (wj >> 1) & 1, wj >> 2
                    col0 = 1 + WIN * (16 * wg2 + wj)
                    nc.tensor.matmul(
                        sp[64 * b1:64 * b1 + 64, 2 * s2 + u, :],
                        qT[64 * r:64 * r + 64, c, col0:col0 + WIN],
                        kT[64 * r:64 * r + 64, c, col0:col0 + WIN],
                        start=True,
                        stop=True,
                    )
                pb = pp.tile([128, 8, WIN], BF16, tag="P", bufs=6, name="pb")
                P_sb[r] = pb
                nc.scalar.activation(
                    pb[:, :, :].rearrange("p a b -> p (a b)"),
                    sp[:, :, :].rearrange("p a b -> p (a b)"),
                    mybir.ActivationFunctionType.Exp,
                    bias=0.0, scale=SCALE)
                sums = stats.tile([128, 8, 1], F32, tag="sums", name="sums")
                nc.vector.reduce_sum(
                    sums[:, :, :], pb[:, :, :], axis=mybir.AxisListType.X,
                    op=mybir.AluOpType.add)
                rs = stats.tile([128, 8, 1], F32, tag="rs", name="rs")
                nc.vector.reciprocal(rs[:, :, :], sums[:, :, :])
                nc.gpsimd.tensor_tensor(
                    pb[:, :, :], pb[:, :, :],
                    rs[:, :, :].broadcast_to([128, 8, WIN]),
                    op=mybir.AluOpType.mult)
            return P_sb

        def win_back(wg2, c, P_sb):
            """P transpose + P@V matmuls + attnT drain for one iteration."""
            PT_ps = pwin.tile([128, 8, 128], BF16, tag="big", name="ptps")
            for r in range(2):
                for s2 in range(4):
                    nc.tensor.transpose(
                        PT_ps[:, 4 * r + s2, :],
                        P_sb[r][:, 2 * s2:2 * s2 + 2, :].rearrange(
                            "p a b -> p (a b)"),
                        ident[:, :])
            PT_sb = pp.tile([128, 8, 128], BF16, tag="PT", bufs=2, name="ptsb")
            nc.vector.tensor_copy(PT_sb[:, 0:4, :], PT_ps[:, 0:4, :])
            nc.vector.tensor_copy(PT_sb[:, 4:8, :], PT_ps[:, 4:8, :])
            O_ps = [None, None]
            for u in range(2):
                op = (pwin if u == 0 else pr64).tile(
                    [128, 8, WIN], F32, tag=("big" if u == 0 else "r64"),
                    name="op")
                O_ps[u] = op
                for b1 in range(2):
                    for s2 in range(4):
                        wj = 4 * s2 + 2 * b1 + u
                        w_abs = 16 * wg2 + wj
                        for r in range(2):
                            h = 2 * c + r
                            nc.tensor.matmul(
                                op[64 * r:64 * r + 64, 2 * s2 + b1, :],
                                v_sb[64 * u:64 * u + 64, w_abs // 2,
                                     64 * h:64 * h + 64],
                                PT_sb[64 * u:64 * u + 64, 4 * r + s2,
                                      64 * b1:64 * b1 + 64],
                                start=True,
                                stop=True,
                            )
            cb = 1 + 1024 * wg2
            av = attnT[:, c, cb:cb + 1024].rearrange(
                "p (a b u q) -> p a b u q", a=4, b=2, u=2)
            for u in range(2):
                eng = nc.vector.tensor_copy if u == 0 else nc.scalar.copy
                eng(av[:, :, :, u, :],
                    O_ps[u][:, :, :].rearrange("p (a b) q -> p a b q", a=4))

        def outproj(tq):
            r0 = 128 * tq
            rows = min(128, T - r0)
            ps = pproj.tile([128, D], F32, tag="proj", name="pso")
            for c in range(CH):
                nc.tensor.matmul(
                    ps[:rows, :],
                    attnT[:, c, r0:r0 + rows],
                    wout_sb[:, c, :],
                    start=(c == 0),
                    stop=(c == CH - 1),
                )
            ob = posb.tile([128, D], F32, tag="osb", name="ob")
            if tq % 2 == 0:
                nc.vector.tensor_copy(ob[:rows, :], ps[:rows, :])
            else:
                nc.scalar.copy(ob[:rows, :], ps[:rows, :])
            nc.sync.dma_start(out=out_d[r0:r0 + rows, :], in_=ob[:rows, :])

        # ---------------- weights ----------------

        def w_qk_slices(jjb):
            """All 4 kc-slices of one 128-col q/k weight block in a single
            DMA, so jb_proj for that block unblocks after ~1us of DMA."""
            ws = xstage.tile([128, KC, 128], F32, tag="wst", bufs=3, name="ws")
            nc.sync.dma_start(
                out=ws[:, :, :],
                in_=wqkv_d[:, 128 * jjb:128 * (jjb + 1)].rearrange(
                    "(kc p) e -> p kc e", p=128))
            # stage on GpSimd (idle early) to keep DVE/ACT free for the
            # x-cast / transpose-drain pipeline
            nc.gpsimd.tensor_copy(
                wqkv_sb[:, :, 128 * jjb:128 * (jjb + 1)], ws[:, :, :])

        def w_v_slice(kc):
            ws = xstage.tile([128, 512], F32, tag="wst", bufs=3, name="wsv")
            nc.sync.dma_start(
                out=ws[:, :], in_=wqkv_d[128 * kc:128 * (kc + 1), 2 * D:3 * D])
            nc.gpsimd.tensor_copy(wqkv_sb[:, kc, 2 * D:3 * D], ws[:, :])

        def w_out_slice(kc):
            ws = xstage.tile([128, 512], F32, tag="wst", bufs=3, name="wso")
            nc.sync.dma_start(
                out=ws[:, :], in_=wout_d[128 * kc:128 * (kc + 1), :])
            if kc % 2 == 0:
                nc.vector.tensor_copy(wout_sb[:, kc, :], ws[:, :])
            else:
                nc.scalar.copy(wout_sb[:, kc, :], ws[:, :])

        # ---------------- the schedule ----------------

        # Prelude: blocks 0,1 projected; q0all/s00/quad 0; v tiles 0..3.
        # All loads are emitted up front in arrival order (the DMA queue is
        # a serial resource AND the PE runs its queue in-order, so PE work
        # must be emitted in the order its inputs land).
        make_identity(nc, ident)
        nc.vector.memset(ones_col[:, :], 1.0)
        nc.vector.memset(ones_row[:, :], 1.0)
        dma_x(0)
        w_qk_slices(0)
        w_qk_slices(4)
        dma_x(1)             # registers half-thunks
        st.pop(("xsh", 0))()
        w_qk_slices(1)
        w_qk_slices(5)
        st.pop(("xsh", 1))()
        for jjb in (2, 6, 3, 7):
            w_qk_slices(jjb)
        for kc in range(KC):
            w_v_slice(kc)
        dma_x(2)
        cast_x(0)
        for jj in range(4):
            transp(0, jj)
        jb_proj(0, 0), jb_proj(0, 4)
        cast_x(1)
        for jj in range(4):
            transp(1, jj)
        for jjb in (1, 5, 2, 6, 3, 7):
            jb_proj(0, jjb)
        build_q0all()
        for jjb in (0, 4, 1, 5, 2, 6, 3, 7):
            jb_proj(1, jjb)
        s00_tok0()
        s0_quad(0)
        cast_x(2)
        v0_proj()
        for vt in range(4):
            v_proj(vt)

        # Window supergroups with projection quanta as filler.  The window
        # pipeline is 3-stage: back(i) is emitted two fronts after front(i),
        # giving the softmax chain (exp -> reduce -> recip -> normalize) two
        # full steps of engine-queue slack before the PT transposes need it.
        pending = []
        ready_oq = []

        def do_back():
            (bg, bc), bP = pending.pop(0)
            win_back(bg, bc, bP)
            if bc == 3:
                # supergroup bg's attnT is final: its outproj tiles (plus
                # the boundary tile it shares with bg-1) become ready
                if bg > 0:
                    ready_oq.append(8 * bg)
                ready_oq.extend(range(8 * bg + 1, 8 * bg + 8))

        def pop_oq(n):
            for _ in range(min(n, len(ready_oq))):
                outproj(ready_oq.pop(0))

        def emit_block(j, cast=True):
            """cast + transposes for one block."""
            if cast:
                cast_x(j)
            if j == NBLK:
                transp(NBLK, 0)  # tail token -> col 512 of block NBLK-1
            else:
                for jj in range(4):
                    transp(j, jj)

        # Supergroup 0: custom interleave.  Fronts lead (their x2-chain
        # inputs land first); v tiles 4..7 ride behind the first front; the
        # first two backs drain before the Bb-block work.
        dma_x(3)
        dma_x(4)
        emit_block(2, cast=False)
        jb_proj(2, 0), jb_proj(2, 4)
        pending.append(((0, 0), win_front(0, 0)))
        for vt in range(4, 8):
            v_proj(vt)
        jb_proj(2, 1), jb_proj(2, 5)
        pending.append(((0, 1), win_front(0, 1)))
        cast_x(3)
        emit_block(3, cast=False)
        jb_proj(2, 2), jb_proj(2, 6)
        jb_proj(2, 3), jb_proj(2, 7)
        pending.append(((0, 2), win_front(0, 2)))
        do_back()
        pending.append(((0, 3), win_front(0, 3)))
        s0_quad(1)
        do_back()
        cast_x(4)
        for vt in range(8, 12):
            v_proj(vt)
        # block-4 transposes here so their drains ride the jb_proj(3) stretch
        emit_block(4, cast=False)
        for jjb in range(4):
            jb_proj(3, jjb)
        for jjb in range(4, 8):
            jb_proj(3, jjb)
        s0_quad(2)
        for kc in range(KC):
            w_out_slice(kc)

        for g in (1, 2):
            A, Bb = 2 * g + 2, 2 * g + 3
            # prefetch DMAs for upcoming blocks (loads lead the queue)
            for jd in (2 * g + 3, 2 * g + 4):
                if jd <= NBLK:
                    dma_x(jd)
            # pre-front quanta: v tiles of block 2g+1 (block A's transposes
            # already ran at the end of the previous supergroup)
            for vt in range(8 * g + 4, 8 * g + 8):
                v_proj(vt)
            if Bb <= NBLK:
                # cast Bb now, while the Pool queue is clear of normalizes
                cast_x(Bb)
            np_ = 1 if g < 2 else 3
            jb_proj(A, 0), jb_proj(A, 4)
            pending.append(((g, 0), win_front(g, 0)))
            if len(pending) > 2:
                do_back()
            jb_proj(A, 1), jb_proj(A, 5)
            pending.append(((g, 1), win_front(g, 1)))
            if Bb <= NBLK:
                # Bb transposes here so their PSUM drains clear DVE/ACT
                # ~3us before jb_proj(Bb) needs xT
                emit_block(Bb, cast=False)
            pop_oq(np_)
            if len(pending) > 2:
                do_back()
            jb_proj(A, 2), jb_proj(A, 6)
            pending.append(((g, 2), win_front(g, 2)))
            pop_oq(np_)
            if len(pending) > 2:
                do_back()
            jb_proj(A, 3), jb_proj(A, 7)
            pending.append(((g, 3), win_front(g, 3)))
            pop_oq(np_)
            if len(pending) > 2:
                do_back()
            s0_quad(2 * g + 1)
            pop_oq(np_)
            cast_x(Bb + 1)  # next supergroup's A block
            for vt in range(8 * g + 8, 8 * g + 12):
                v_proj(vt)
            pop_oq(2)
            # next supergroup's A-block transposes (tail token for g==2)
            # here so their drains ride the jb_proj(Bb) stretch
            emit_block(2 * g + 4, cast=False)
            for jjb in range(4):
                jb_proj(Bb, jjb)
            pop_oq(1)
            for jjb in range(4, 8):
                jb_proj(Bb, jjb)
            s0_quad(2 * g + 2)
            pop_oq(1)

        # Supergroup 3: the jb(8) filler is ~free (single tail token), so
        # drain the two pending backs up front to unlock outproj filler,
        # and spread the global-token chain between fronts.
        do_back()
        do_back()
        for vt in range(28, 32):
            v_proj(vt)
        jb_proj(8, 0), jb_proj(8, 4)
        pending.append(((3, 0), win_front(3, 0)))
        pop_oq(2)
        jb_proj(8, 1), jb_proj(8, 5)
        pending.append(((3, 1), win_front(3, 1)))
        pop_oq(2)
        jb_proj(8, 2), jb_proj(8, 6)
        pending.append(((3, 2), win_front(3, 2)))
        do_back()
        pop_oq(2)
        jb_proj(8, 3), jb_proj(8, 7)
        pending.append(((3, 3), win_front(3, 3)))
        s0_quad(7)
        pop_oq(2)
        do_back()
        z_sum()
        z_fin()
        pop_oq(2)
        o0_accum()

        # Tail: drain the window pipeline, then remaining output tiles.
        # Tile 0 (global token) goes first so the final store is the tiny
        # single-row tile TQ-1.
        while pending:
            do_back()
            pop_oq(2)
        # alternate drain parity (DVE/ACT) through the tail; tiny tile TQ-1
        # stores last
        tail = [0] + [t for p in zip(ready_oq[1::2], ready_oq[0::2])
                      for t in p] + [TQ - 1]
        ready_oq[:] = tail
        pop_oq(len(ready_oq))


def build(T=T_FULL):
    nc = bacc.Bacc("TRN2", target_bir_lowering=False, debug=False,
                   num_devices=N_CORES)
    x_d = nc.dram_tensor("x", [T, D], F32, kind="ExternalInput")
    wqkv_d = nc.dram_tensor("w_qkv", [D, 3 * D], F32, kind="ExternalInput")
    wout_d = nc.dram_tensor("w_out", [D, D], F32, kind="ExternalInput")
    out_d = nc.dram_tensor("out", [T, D], F32, kind="ExternalOutput")
    with tile.TileContext(nc) as tc:
        _emit(nc, tc, x_d.ap(), wqkv_d.ap(), wout_d.ap(), out_d.ap(), T)
    nc.compile()
    return nc


_NC_CACHE = {}


def kernel(x, w_qkv, w_out):
    x = np.ascontiguousarray(np.asarray(x, dtype=np.float32))
    w_qkv = np.ascontiguousarray(np.asarray(w_qkv, dtype=np.float32))
    w_out = np.ascontiguousarray(np.asarray(w_out, dtype=np.float32))
    assert x.shape == (B, T_FULL, D)

    if "nc" not in _NC_CACHE:
        _NC_CACHE["nc"] = build(T_FULL)
    nc = _NC_CACHE["nc"]

    in_maps = [
        {"x": x[b], "w_qkv": w_qkv, "w_out": w_out} for b in range(N_CORES)
    ]
    last_err = None
    for _attempt in range(4):
        try:
            res = run_bass_kernel_spmd(nc, in_maps, core_ids=list(range(N_CORES)))
            break
        except Exception as e:  # transient NRT device errors
            last_err = e
            try:  # force a fresh PJRT client before retrying
                import jax
                jax.clear_caches()
                jax.extend.backend.clear_backends()
            except Exception:
                pass
            import time as _time
            _time.sleep(5)
    else:
        raise last_err
    return np.stack([res.results[b]["out"] for b in range(N_CORES)], axis=0)



# revision 80
# speedup vs baseline: 1.0043x; 1.0043x over previous
"""BBox window attention kernel for 8 TRN2 NeuronCores — streaming schedule.

Sharding: data-parallel over batch B=8 -> one batch element per core.
Each core computes the full attention for its batch element; no collectives.

v3: single streaming pipeline (v2) plus a token-major global-token path.
x is loaded per 512-token block; each block's cast/transpose/qkv-projection
work is emitted as small "filler quanta" interleaved between window-attention
front/back steps, so the per-iteration softmax chain (exp -> reduce -> recip
-> gpsimd normalize) hides behind projection matmuls and the PE never
starves.  Matmul cost is priced by output free-size only, so the global
token's scores are computed token-major ([128 tok, 8 head] via N=8 matmuls,
the exp of which IS P0^T — no transposes) and out0 accumulates feature-major
([128 feat, 2 head] via N=2 matmuls); softmax denominators come from
ones-column matmuls, their broadcast from a PE outer product.  Output
projection tiles of supergroup g ride as filler inside supergroup g+1; the
tail supergroup leads with quad7/z so their cross-engine latency hides under
the entry backs.

Per-core math (all matmuls bf16 with f32 PSUM accumulation): feature-major
q/k, token-major v (shifted by 1), global token via exp without
max-subtraction, windows in 16-window supergroups with PSUM tile_position
row discipline (row-0 pools vs row-64 pool).
"""

import sys

for _p in ("/opt/trn_rl_repo",):
    if _p not in sys.path:
        sys.path.insert(0, _p)

import numpy as np

import concourse.bass as bass
import concourse.tile as tile
from concourse import bacc, mybir
from concourse.bass_utils import run_bass_kernel_spmd
from concourse.masks import make_identity

F32 = mybir.dt.float32
BF16 = mybir.dt.bfloat16

B, T_FULL, D = 8, 4097, 512
H, WIN, d_head = 8, 64, 64
N_CORES = 8
CH = 4          # head-pair chunks (128 features each)
KC = 4          # contraction chunks of 128 over D
BLK = 512       # token block size (one PSUM bank at f32)
SCALE = float(d_head) ** -0.5


def _emit(nc, tc, x_d, wqkv_d, wout_d, out_d, T):
    TW = T - 1                 # window tokens
    NW = TW // WIN             # number of windows (64)
    assert NW % 16 == 0
    WG2 = NW // 16             # supergroups of 16 windows (4)
    NBLK = TW // BLK           # 8 full blocks; block NBLK is the 1-token tail
    VT = TW // 128             # v tiles (tokens 1..TW)
    TQ = (T + 127) // 128      # output tiles of 128 tokens

    def pool(name, **kw):
        return tc.tile_pool(name=name, **kw)

    with pool("persist", bufs=1) as persist, \
         pool("xstage", bufs=2) as xstage, \
         pool("stats", bufs=4) as stats, \
         pool("pp", bufs=4) as pp, \
         pool("osb", bufs=5) as posb, \
         pool("psum_w0", bufs=3, space="PSUM") as pwin, \
         pool("psum_pr", bufs=3, space="PSUM") as pproj, \
         pool("psum_r64", bufs=2, space="PSUM") as pr64:

        # PSUM discipline (hardware-validated): all matmul groups landing in
        # one physical bank must share the same tile_position ROW (= lhsT/rhs
        # partition base).  pwin/pproj host row-0 groups only; pr64 hosts
        # row-64 groups (odd head-half S tiles / odd window-parity O tiles).

        ident = persist.tile([128, 128], BF16)

        wqkv_sb = persist.tile([128, KC, 3 * D], BF16)
        wout_sb = persist.tile([128, KC, D], BF16)
        qT = persist.tile([128, CH, T], BF16)
        kT = persist.tile([128, CH, T], BF16)
        v_sb = persist.tile([128, VT, D], BF16)
        v0_sb = persist.tile([1, D], BF16)
        q0all = persist.tile([128, CH, 8], BF16)
        P0T_sb = persist.tile([128, VT, 8], BF16)
        p00_sb = persist.tile([1, 8], BF16)
        ones_col = persist.tile([128, 1], BF16)
        ones_row = persist.tile([1, 128], BF16)
        zinv_sb = persist.tile([1, 8], F32)
        zinv_bf = persist.tile([1, 8], BF16)
        zb_sb = persist.tile([128, 8], F32)
        attnT = persist.tile([128, CH, T], BF16)

        st = {}  # per-block tile handles

        # ---------------- projection quanta ----------------

        def dma_x(j):
            if j < NBLK:
                xs = xstage.tile([128, 4, BLK], F32, tag="xs", name=f"xs{j}")
                if j == 0:
                    # quarters so block 0's cast/transposes start ~2.2us earlier
                    for qq in range(4):
                        nc.sync.dma_start(
                            out=xs[:, qq:qq + 1, :],
                            in_=x_d[128 * qq:128 * (qq + 1), :].rearrange(
                                "(a p) e -> p a e", p=128))
                elif j == 1:
                    # halves, interleaved into the w_qkv DMA stream
                    st[("xsh", 0)] = lambda xs=xs: nc.sync.dma_start(
                        out=xs[:, 0:2, :],
                        in_=x_d[BLK:BLK + 256, :].rearrange(
                            "(a p) e -> p a e", p=128))
                    st[("xsh", 1)] = lambda xs=xs: nc.sync.dma_start(
                        out=xs[:, 2:4, :],
                        in_=x_d[BLK + 256:2 * BLK, :].rearrange(
                            "(a p) e -> p a e", p=128))
                else:
                    nc.sync.dma_start(
                        out=xs[:, :, :],
                        in_=x_d[BLK * j:BLK * (j + 1), :].rearrange(
                            "(a p) e -> p a e", p=128),
                    )
            else:  # tail: token T-1 (shares the weight-staging slots)
                xs = xstage.tile([1, D], F32, tag="wst", bufs=3, name="xs_t")
                nc.sync.dma_start(out=xs[:, :], in_=x_d[T - 1:T, :])
            st[("xs", j)] = xs

        def cast_x(j):
            # f32 -> bf16 cast.  Prelude blocks (0-2) go on DVE/ACT (idle
            # there); later blocks go on GpSimd in two halves, keeping
            # DVE/ACT free for PSUM drains while Pool normalizes slot in
            # between the halves.
            xs = st.pop(("xs", j))
            if j < NBLK:
                xc = xstage.tile([128, 4, BLK], BF16, tag="xc", name=f"xc{j}")
                if j == 0:
                    for qq in range(4):
                        nc.vector.tensor_copy(xc[:, qq, :], xs[:, qq, :])
                elif j == 1:
                    nc.vector.tensor_copy(xc[:, 0:2, :], xs[:, 0:2, :])
                    nc.vector.tensor_copy(xc[:, 2:4, :], xs[:, 2:4, :])
                elif j == 2:
                    # prelude gap: DVE+ACT halves run in parallel while the
                    # window pipeline hasn't started yet
                    nc.vector.tensor_copy(xc[:, 0:2, :], xs[:, 0:2, :])
                    nc.scalar.copy(xc[:, 2:4, :], xs[:, 2:4, :])
                else:
                    nc.gpsimd.tensor_copy(xc[:, 0:2, :], xs[:, 0:2, :])
                    nc.gpsimd.tensor_copy(xc[:, 2:4, :], xs[:, 2:4, :])
            else:
                xc = xstage.tile([1, D], BF16, tag="xc", name="xc_t")
                nc.vector.tensor_copy(xc[:, :], xs[:, :])
            st[("xc", j)] = xc

        def transp(j, jj):
            """Transpose token tile jj (128 tokens) of block j into xT(j).

            xT blocks have 513 columns: col 512 (= next block's first token)
            is written by the next block's jj=0 call, so v tiles never span
            two xT tiles.
            """
            if j == NBLK:  # tail token: fills col 512 of block NBLK-1 only
                xc = st[("xc", j)]
                # inner dim 2 keeps each kc-slice 4-byte aligned in PSUM
                # (walrus requires 4B-aligned matmul outputs)
                tp = pproj.tile([128, KC, 2], BF16, tag="proj", name="tp_t")
                for kc in range(KC):
                    nc.tensor.transpose(
                        tp[:, kc, 0:1], xc[:, 128 * kc:128 * (kc + 1)],
                        ident[0:1, 0:1])
                nc.vector.tensor_copy(st[("xT", NBLK - 1)][:, :, BLK:BLK + 1],
                                      tp[:, :, 0:1])
                return
            xc = st[("xc", j)]
            if jj == 0:
                xT = xstage.tile([128, KC, BLK + 1], BF16, tag="xT",
                                 bufs=2, name=f"xT{j}")
                st[("xT", j)] = xT
            xT = st[("xT", j)]
            tp = pproj.tile([128, KC, 128], BF16, tag="proj", name="tp")
            for kc in range(KC):
                nc.tensor.transpose(
                    tp[:, kc, :], xc[:, jj, 128 * kc:128 * (kc + 1)],
                    ident[:, :])
            dst = xT[:, :, 128 * jj:128 * (jj + 1)]
            if jj % 2 == 0:
                nc.vector.tensor_copy(dst, tp[:, :, :])
            else:
                nc.scalar.copy(dst, tp[:, :, :])
            if jj == 0 and j > 0:
                # previous block's overlap column (token BLK*j)
                nc.vector.tensor_copy(st[("xT", j - 1)][:, :, BLK:BLK + 1],
                                      tp[:, :, 0:1])

        def jb_proj(j, jjb):
            """q/k feature block jjb (0..3 -> qT chunk, 4..7 -> kT chunk)."""
            c0 = BLK * j
            w = min(BLK, T - c0)
            ps = pproj.tile([128, BLK], F32, tag="proj", name="psjb")
            for kc in range(KC):
                if j < NBLK:
                    rhs = st[("xT", j)][:, kc, 0:w]
                else:  # tail token lives in block NBLK-1's overlap column
                    rhs = st[("xT", NBLK - 1)][:, kc, BLK:BLK + w]
                nc.tensor.matmul(
                    ps[:, :w],
                    wqkv_sb[:, kc, 128 * jjb:128 * (jjb + 1)],
                    rhs,
                    start=(kc == 0),
                    stop=(kc == KC - 1),
                )
            if jjb < 4:
                dst = qT[:, jjb, c0:c0 + w]
            else:
                dst = kT[:, jjb - 4, c0:c0 + w]
            if jjb % 2 == 0:
                nc.vector.tensor_copy(dst, ps[:, :w])
            else:
                nc.scalar.copy(dst, ps[:, :w])

        def v_proj(vt):
            """v tile vt: tokens 1+128vt .. 129+128vt (within xT block a)."""
            a = (128 * vt) // BLK
            off = 1 + 128 * vt - BLK * a
            xT = st[("xT", a)]
            ps = pproj.tile([128, D], F32, tag="proj", name="psv")
            for kc in range(KC):
                nc.tensor.matmul(
                    ps[:, :],
                    xT[:, kc, off:off + 128],
                    wqkv_sb[:, kc, 2 * D:3 * D],
                    start=(kc == 0),
                    stop=(kc == KC - 1),
                )
            if vt % 2 == 0:
                nc.vector.tensor_copy(v_sb[:, vt, :], ps[:, :])
            else:
                nc.scalar.copy(v_sb[:, vt, :], ps[:, :])

        def v0_proj():
            xT = st[("xT", 0)]
            ps = pproj.tile([1, D], F32, tag="proj", name="psv0")
            for kc in range(KC):
                nc.tensor.matmul(
                    ps[:, :], xT[:, kc, 0:1], wqkv_sb[:, kc, 2 * D:3 * D],
                    start=(kc == 0), stop=(kc == KC - 1))
            nc.vector.tensor_copy(v0_sb[:, :], ps[:, :])

        def build_q0all():
            # q0all column h holds q0 of head h only in head h's partition
            # range of its chunk and zeros elsewhere, so the four chunk
            # matmuls of s0 accumulate cleanly.
            nc.vector.memset(q0all[:, :, :], 0.0)
            for h in range(H):
                r0 = 64 * (h % 2)
                nc.vector.tensor_copy(
                    q0all[r0:r0 + 64, h // 2, h:h + 1],
                    qT[r0:r0 + 64, h // 2, 0:1])

        def s00_tok0():
            """Global-token score vs token 0, token-major: out [1, 8]."""
            ps = pproj.tile([1, 8], F32, tag="proj", name="ps00")
            for c in range(CH):
                nc.tensor.matmul(ps[:, :], kT[:, c, 0:1], q0all[:, c, :],
                                 start=(c == 0), stop=(c == CH - 1))
            nc.scalar.activation(p00_sb[:, :], ps[:, :],
                                 mybir.ActivationFunctionType.Exp,
                                 bias=0.0, scale=SCALE)

        def s0_quad(qd):
            """Global-token probs for 4 token tiles (128 tokens each),
            token-major: matmul N=8 per (tile, c) instead of N=512, and the
            exp'd result IS P0T (no transposes, no overlap columns)."""
            ps = pproj.tile([128, 4, 8], F32, tag="proj", name=f"ps0q{qd}")
            for i in range(4):
                t0 = 1 + 128 * (4 * qd + i)
                for c in range(CH):
                    nc.tensor.matmul(ps[:, i, :], kT[:, c, t0:t0 + 128],
                                     q0all[:, c, :],
                                     start=(c == 0), stop=(c == CH - 1))
            nc.scalar.activation(
                P0T_sb[:, 4 * qd:4 * qd + 4, :].rearrange("p a b -> p (a b)"),
                ps[:, :, :].rearrange("p a b -> p (a b)"),
                mybir.ActivationFunctionType.Exp, bias=0.0, scale=SCALE)

        def z_sum():
            """Softmax denominator part 1: ones-matmul over P0T tiles, then
            recip+cast on DVE (give PE filler before calling z_fin)."""
            zps = pproj.tile([1, 8], F32, tag="proj", name="zps")
            nc.tensor.matmul(zps[:, :], ones_col[0:1, 0:1], p00_sb[:, :],
                             start=True, stop=False)
            for vt in range(VT):
                nc.tensor.matmul(zps[:, :], ones_col[:, :], P0T_sb[:, vt, :],
                                 start=False, stop=(vt == VT - 1))
            nc.vector.reciprocal(zinv_sb[:, :], zps[:, :])
            nc.vector.tensor_copy(zinv_bf[:, :], zinv_sb[:, :])

        def z_fin():
            """Part 2: partition-broadcast 1/Z (outer product with ones)."""
            zbps = pproj.tile([128, 8], F32, tag="proj", name="zbps")
            nc.tensor.matmul(zbps[:, :], ones_row[:, :], zinv_bf[:, :],
                             start=True, stop=True)
            nc.vector.tensor_copy(zb_sb[:, :], zbps[:, :])

        def o0_accum():
            """out0 via feature-major accumulation: lhsT = v tile (tokens on
            partitions), rhs = P0T column pair -> [128 feats, 2] per chunk;
            N=2 matmuls make this ~free on the PE."""
            ops = pproj.tile([128, 4, 2], F32, tag="proj", name="o0ps")
            for c in range(CH):
                nc.tensor.matmul(ops[:, c, :],
                                 v0_sb[0:1, 128 * c:128 * (c + 1)],
                                 p00_sb[0:1, 2 * c:2 * c + 2],
                                 start=True, stop=False)
                for vt in range(VT):
                    nc.tensor.matmul(ops[:, c, :],
                                     v_sb[:, vt, 128 * c:128 * (c + 1)],
                                     P0T_sb[:, vt, 2 * c:2 * c + 2],
                                     start=False, stop=(vt == VT - 1))
            # normalize + scatter into attnT column 0 (diagonal strips)
            for c in range(CH):
                for r in range(2):
                    h = 2 * c + r
                    nc.scalar.activation(
                        attnT[64 * r:64 * r + 64, c, 0:1],
                        ops[64 * r:64 * r + 64, c, r:r + 1],
                        mybir.ActivationFunctionType.Identity,
                        bias=0.0, scale=zb_sb[64 * r:64 * r + 64, h:h + 1])

        # ---------------- window attention ----------------
        # Window wj (0..15 within a 16-window supergroup) maps to bits
        # (u, b1, s2) = (wj&1, (wj>>1)&1, wj>>2 in 0..3).  Layouts keep
        # every matmul's lhsT/rhs partition base equal and the
        # tile_position row fixed per PSUM tile (hardware requirement):
        #   S tile (per head-half r):  [64*b1 + q, slot=2*s2+u, k]
        #   PT (transposed P):         [64*u + k, slab=4*r+s2, 64*b1 + q]
        #   O tile (per parity u):     [64*r + e, slot=2*s2+b1, q]

        def win_front(wg2, c):
            """S matmuls + softmax for one iteration; returns P tiles."""
            P_sb = [None, None]
            for r in range(2):
                sp = (pwin if r == 0 else pr64).tile(
                    [128, 8, WIN], F32, tag=("big" if r == 0 else "r64"),
                    name="sp")
                for wj in range(16):
                    u, b1, s2 = wj & 1, (wj >> 1) & 1, wj >> 2
                    col0 = 1 + WIN * (16 * wg2 + wj)
                    nc.tensor.matmul(
                        sp[64 * b1:64 * b1 + 64, 2 * s2 + u, :],
                        qT[64 * r:64 * r + 64, c, col0:col0 + WIN],
                        kT[64 * r:64 * r + 64, c, col0:col0 + WIN],
                        start=True,
                        stop=True,
                    )
                pb = pp.tile([128, 8, WIN], BF16, tag="P", bufs=6, name="pb")
                P_sb[r] = pb
                nc.scalar.activation(
                    pb[:, :, :].rearrange("p a b -> p (a b)"),
                    sp[:, :, :].rearrange("p a b -> p (a b)"),
                    mybir.ActivationFunctionType.Exp,
                    bias=0.0, scale=SCALE)
                sums = stats.tile([128, 8, 1], F32, tag="sums", name="sums")
                nc.vector.reduce_sum(
                    sums[:, :, :], pb[:, :, :], axis=mybir.AxisListType.X,
                    op=mybir.AluOpType.add)
                rs = stats.tile([128, 8, 1], F32, tag="rs", name="rs")
                nc.vector.reciprocal(rs[:, :, :], sums[:, :, :])
                nc.gpsimd.tensor_tensor(
                    pb[:, :, :], pb[:, :, :],
                    rs[:, :, :].broadcast_to([128, 8, WIN]),
                    op=mybir.AluOpType.mult)
            return P_sb

        def win_back(wg2, c, P_sb):
            """P transpose + P@V matmuls + attnT drain for one iteration."""
            PT_ps = pwin.tile([128, 8, 128], BF16, tag="big", name="ptps")
            for r in range(2):
                for s2 in range(4):
                    nc.tensor.transpose(
                        PT_ps[:, 4 * r + s2, :],
                        P_sb[r][:, 2 * s2:2 * s2 + 2, :].rearrange(
                            "p a b -> p (a b)"),
                        ident[:, :])
            PT_sb = pp.tile([128, 8, 128], BF16, tag="PT", bufs=2, name="ptsb")
            nc.vector.tensor_copy(PT_sb[:, 0:4, :], PT_ps[:, 0:4, :])
            nc.vector.tensor_copy(PT_sb[:, 4:8, :], PT_ps[:, 4:8, :])
            O_ps = [None, None]
            for u in range(2):
                op = (pwin if u == 0 else pr64).tile(
                    [128, 8, WIN], F32, tag=("big" if u == 0 else "r64"),
                    name="op")
                O_ps[u] = op
                for b1 in range(2):
                    for s2 in range(4):
                        wj = 4 * s2 + 2 * b1 + u
                        w_abs = 16 * wg2 + wj
                        for r in range(2):
                            h = 2 * c + r
                            nc.tensor.matmul(
                                op[64 * r:64 * r + 64, 2 * s2 + b1, :],
                                v_sb[64 * u:64 * u + 64, w_abs // 2,
                                     64 * h:64 * h + 64],
                                PT_sb[64 * u:64 * u + 64, 4 * r + s2,
                                      64 * b1:64 * b1 + 64],
                                start=True,
                                stop=True,
                            )
            cb = 1 + 1024 * wg2
            av = attnT[:, c, cb:cb + 1024].rearrange(
                "p (a b u q) -> p a b u q", a=4, b=2, u=2)
            for u in range(2):
                eng = nc.vector.tensor_copy if u == 0 else nc.scalar.copy
                eng(av[:, :, :, u, :],
                    O_ps[u][:, :, :].rearrange("p (a b) q -> p a b q", a=4))

        def outproj(tq):
            r0 = 128 * tq
            rows = min(128, T - r0)
            ps = pproj.tile([128, D], F32, tag="proj", name="pso")
            for c in range(CH):
                nc.tensor.matmul(
                    ps[:rows, :],
                    attnT[:, c, r0:r0 + rows],
                    wout_sb[:, c, :],
                    start=(c == 0),
                    stop=(c == CH - 1),
                )
            ob = posb.tile([128, D], F32, tag="osb", name="ob")
            if tq % 2 == 0:
                nc.vector.tensor_copy(ob[:rows, :], ps[:rows, :])
            else:
                nc.scalar.copy(ob[:rows, :], ps[:rows, :])
            nc.sync.dma_start(out=out_d[r0:r0 + rows, :], in_=ob[:rows, :])

        def outproj_pre(tq):
            """Chunks 0..2 of a tail tile, PSUM held open: runs between the
            last two backs while chunk 3's attnT is still in flight."""
            r0 = 128 * tq
            ps = pproj.tile([128, D], F32, tag="proj", name=f"psop{tq}")
            for c in range(CH - 1):
                nc.tensor.matmul(ps[:, :], attnT[:, c, r0:r0 + 128],
                                 wout_sb[:, c, :],
                                 start=(c == 0), stop=False)
            st[("pso", tq)] = ps

        def outproj_fin(tq):
            r0 = 128 * tq
            ps = st.pop(("pso", tq))
            nc.tensor.matmul(ps[:, :], attnT[:, CH - 1, r0:r0 + 128],
                             wout_sb[:, CH - 1, :], start=False, stop=True)
            ob = posb.tile([128, D], F32, tag="osb", name="ob")
            if tq % 2 == 0:
                nc.vector.tensor_copy(ob[:, :], ps[:, :])
            else:
                nc.scalar.copy(ob[:, :], ps[:, :])
            nc.sync.dma_start(out=out_d[r0:r0 + 128, :], in_=ob[:, :])

        # ---------------- weights ----------------

        def w_qk_slices(jjb):
            """All 4 kc-slices of one 128-col q/k weight block in a single
            DMA, so jb_proj for that block unblocks after ~1us of DMA."""
            ws = xstage.tile([128, KC, 128], F32, tag="wst", bufs=3, name="ws")
            nc.sync.dma_start(
                out=ws[:, :, :],
                in_=wqkv_d[:, 128 * jjb:128 * (jjb + 1)].rearrange(
                    "(kc p) e -> p kc e", p=128))
            # stage on GpSimd (idle early) to keep DVE/ACT free for the
            # x-cast / transpose-drain pipeline; two halves so jb_proj's
            # kc=0 accumulation step unblocks earlier
            nc.gpsimd.tensor_copy(
                wqkv_sb[:, 0:2, 128 * jjb:128 * (jjb + 1)], ws[:, 0:2, :])
            nc.gpsimd.tensor_copy(
                wqkv_sb[:, 2:4, 128 * jjb:128 * (jjb + 1)], ws[:, 2:4, :])

        def w_v_slice(kc):
            ws = xstage.tile([128, 512], F32, tag="wst", bufs=3, name="wsv")
            nc.sync.dma_start(
                out=ws[:, :], in_=wqkv_d[128 * kc:128 * (kc + 1), 2 * D:3 * D])
            nc.gpsimd.tensor_copy(wqkv_sb[:, kc, 2 * D:3 * D], ws[:, :])

        def w_out_slice(kc):
            ws = xstage.tile([128, 512], F32, tag="wst", bufs=3, name="wso")
            nc.sync.dma_start(
                out=ws[:, :], in_=wout_d[128 * kc:128 * (kc + 1), :])
            # Pool: its g0-end queue is clear, and this keeps DVE/ACT free
            # right when g1's first jb/outproj quanta need their drains
            nc.gpsimd.tensor_copy(wout_sb[:, kc, :], ws[:, :])

        # ---------------- the schedule ----------------

        # Prelude: blocks 0,1 projected; q0all/s00/quad 0; v tiles 0..3.
        # All loads are emitted up front in arrival order (the DMA queue is
        # a serial resource AND the PE runs its queue in-order, so PE work
        # must be emitted in the order its inputs land).
        make_identity(nc, ident)
        nc.vector.memset(ones_col[:, :], 1.0)
        nc.vector.memset(ones_row[:, :], 1.0)
        dma_x(0)
        w_qk_slices(0)
        w_qk_slices(4)
        dma_x(1)             # registers half-thunks
        st.pop(("xsh", 0))()
        w_qk_slices(1)
        w_qk_slices(5)
        st.pop(("xsh", 1))()
        for jjb in (2, 6, 3, 7):
            w_qk_slices(jjb)
        for kc in range(KC):
            w_v_slice(kc)
        dma_x(2)
        cast_x(0)
        for jj in range(4):
            transp(0, jj)
        jb_proj(0, 0), jb_proj(0, 4)
        cast_x(1)
        for jj in range(4):
            transp(1, jj)
        for jjb in (1, 5, 2, 6, 3, 7):
            jb_proj(0, jjb)
        build_q0all()
        for jjb in (0, 4, 1, 5, 2, 6, 3, 7):
            jb_proj(1, jjb)
        s00_tok0()
        s0_quad(0)
        cast_x(2)
        v0_proj()
        for vt in range(4):
            v_proj(vt)

        # Window supergroups with projection quanta as filler.  The window
        # pipeline is 3-stage: back(i) is emitted two fronts after front(i),
        # giving the softmax chain (exp -> reduce -> recip -> normalize) two
        # full steps of engine-queue slack before the PT transposes need it.
        pending = []
        ready_oq = []

        def do_back():
            (bg, bc), bP = pending.pop(0)
            win_back(bg, bc, bP)
            if bc == 3:
                # supergroup bg's attnT is final: its outproj tiles (plus
                # the boundary tile it shares with bg-1) become ready
                if bg > 0:
                    ready_oq.append(8 * bg)
                ready_oq.extend(range(8 * bg + 1, 8 * bg + 8))

        def pop_oq(n):
            for _ in range(min(n, len(ready_oq))):
                outproj(ready_oq.pop(0))

        def emit_block(j, cast=True):
            """cast + transposes for one block."""
            if cast:
                cast_x(j)
            if j == NBLK:
                transp(NBLK, 0)  # tail token -> col 512 of block NBLK-1
            else:
                for jj in range(4):
                    transp(j, jj)

        # Supergroup 0: custom interleave.  Fronts lead (their x2-chain
        # inputs land first); v tiles 4..7 ride behind the first front; the
        # first two backs drain before the Bb-block work.
        dma_x(3)
        dma_x(4)
        emit_block(2, cast=False)
        jb_proj(2, 0), jb_proj(2, 4)
        pending.append(((0, 0), win_front(0, 0)))
        for vt in range(4, 8):
            v_proj(vt)
        jb_proj(2, 1), jb_proj(2, 5)
        pending.append(((0, 1), win_front(0, 1)))
        cast_x(3)
        emit_block(3, cast=False)
        jb_proj(2, 2), jb_proj(2, 6)
        jb_proj(2, 3), jb_proj(2, 7)
        pending.append(((0, 2), win_front(0, 2)))
        do_back()
        pending.append(((0, 3), win_front(0, 3)))
        s0_quad(1)
        do_back()
        cast_x(4)
        for vt in range(8, 12):
            v_proj(vt)
        # block-4 transposes here so their drains ride the jb_proj(3) stretch
        emit_block(4, cast=False)
        for jjb in range(4):
            jb_proj(3, jjb)
        for jjb in range(4, 8):
            jb_proj(3, jjb)
        s0_quad(2)
        for kc in range(KC):
            w_out_slice(kc)

        for g in (1, 2):
            A, Bb = 2 * g + 2, 2 * g + 3
            # prefetch DMAs for upcoming blocks (loads lead the queue)
            for jd in (2 * g + 3, 2 * g + 4):
                if jd <= NBLK:
                    dma_x(jd)
            # pre-front quanta: v tiles of block 2g+1 (block A's transposes
            # already ran at the end of the previous supergroup)
            for vt in range(8 * g + 4, 8 * g + 8):
                v_proj(vt)
            if Bb <= NBLK:
                # cast Bb now, while the Pool queue is clear of normalizes
                cast_x(Bb)
            np_ = 1 if g < 2 else 3
            jb_proj(A, 0), jb_proj(A, 4)
            pending.append(((g, 0), win_front(g, 0)))
            if len(pending) > 2:
                do_back()
            jb_proj(A, 1), jb_proj(A, 5)
            pending.append(((g, 1), win_front(g, 1)))
            if Bb <= NBLK:
                # Bb transposes here so their PSUM drains clear DVE/ACT
                # ~3us before jb_proj(Bb) needs xT
                emit_block(Bb, cast=False)
            pop_oq(np_)
            if len(pending) > 2:
                do_back()
            jb_proj(A, 2), jb_proj(A, 6)
            pending.append(((g, 2), win_front(g, 2)))
            pop_oq(np_)
            if len(pending) > 2:
                do_back()
            jb_proj(A, 3), jb_proj(A, 7)
            pending.append(((g, 3), win_front(g, 3)))
            pop_oq(np_)
            if len(pending) > 2:
                do_back()
            s0_quad(2 * g + 1)
            pop_oq(np_)
            cast_x(Bb + 1)  # next supergroup's A block
            for vt in range(8 * g + 8, 8 * g + 12):
                v_proj(vt)
            pop_oq(2)
            # next supergroup's A-block transposes (tail token for g==2)
            # here so their drains ride the jb_proj(Bb) stretch
            emit_block(2 * g + 4, cast=False)
            for jjb in range(4):
                jb_proj(Bb, jjb)
            pop_oq(1)
            for jjb in range(4, 8):
                jb_proj(Bb, jjb)
            s0_quad(2 * g + 2)
            pop_oq(1)

        # Supergroup 3: the jb(8) filler is ~free (single tail token), so
        # drain the two pending backs up front to unlock outproj filler,
        # and spread the global-token chain between fronts.
        do_back()
        do_back()
        for vt in range(28, 32):
            v_proj(vt)
        jb_proj(8, 0), jb_proj(8, 4)
        pending.append(((3, 0), win_front(3, 0)))
        pop_oq(2)
        jb_proj(8, 1), jb_proj(8, 5)
        pending.append(((3, 1), win_front(3, 1)))
        pop_oq(2)
        jb_proj(8, 2), jb_proj(8, 6)
        pending.append(((3, 2), win_front(3, 2)))
        do_back()
        pop_oq(2)
        jb_proj(8, 3), jb_proj(8, 7)
        pending.append(((3, 3), win_front(3, 3)))
        s0_quad(7)
        pop_oq(2)
        do_back()
        z_sum()
        z_fin()
        pop_oq(2)
        o0_accum()

        # Tail: drain the window pipeline, then remaining output tiles.
        # Tile 0 (global token) goes first so the final store is the tiny
        # single-row tile TQ-1.
        while pending:
            do_back()
            pop_oq(2)
        # alternate drain parity (DVE/ACT) through the tail; tiny tile TQ-1
        # stores last
        tail = [0] + [t for p in zip(ready_oq[1::2], ready_oq[0::2])
                      for t in p] + [TQ - 1]
        ready_oq[:] = tail
        pop_oq(len(ready_oq))


def build(T=T_FULL):
    nc = bacc.Bacc("TRN2", target_bir_lowering=False, debug=False,
                   num_devices=N_CORES)
    x_d = nc.dram_tensor("x", [T, D], F32, kind="ExternalInput")
    wqkv_d = nc.dram_tensor("w_qkv", [D, 3 * D], F32, kind="ExternalInput")
    wout_d = nc.dram_tensor("w_out", [D, D], F32, kind="ExternalInput")
    out_d = nc.dram_tensor("out", [T, D], F32, kind="ExternalOutput")
    with tile.TileContext(nc) as tc:
        _emit(nc, tc, x_d.ap(), wqkv_d.ap(), wout_d.ap(), out_d.ap(), T)
    nc.compile()
    return nc


_NC_CACHE = {}


def kernel(x, w_qkv, w_out):
    x = np.ascontiguousarray(np.asarray(x, dtype=np.float32))
    w_qkv = np.ascontiguousarray(np.asarray(w_qkv, dtype=np.float32))
    w_out = np.ascontiguousarray(np.asarray(w_out, dtype=np.float32))
    assert x.shape == (B, T_FULL, D)

    if "nc" not in _NC_CACHE:
        _NC_CACHE["nc"] = build(T_FULL)
    nc = _NC_CACHE["nc"]

    in_maps = [
        {"x": x[b], "w_qkv": w_qkv, "w_out": w_out} for b in range(N_CORES)
    ]
    last_err = None
    for _attempt in range(4):
        try:
            res = run_bass_kernel_spmd(nc, in_maps, core_ids=list(range(N_CORES)))
            break
        except Exception as e:  # transient NRT device errors
            last_err = e
            try:  # force a fresh PJRT client before retrying
                import jax
                jax.clear_caches()
                jax.extend.backend.clear_backends()
            except Exception:
                pass
            import time as _time
            _time.sleep(5)
    else:
        raise last_err
    return np.stack([res.results[b]["out"] for b in range(N_CORES)], axis=0)

